# revision 8
# baseline (speedup 1.0000x reference)
"""GRU encoder kernel for Trainium2 (8 NeuronCores, batch-sharded).

Problem: embedding lookup [B,T] -> [B,T,E]; xproj = emb @ kernel + bias;
GRU scan over T steps (Keras gate order [z, r, h], reset_after=False);
returns (output [B,T,U], state [B,U]).

Sharding: data-parallel over batch. Each of the 8 cores gets B/8 = 8
batch rows and runs gather + xproj + the full sequential scan for its
slice. No cross-core communication.

Per-core layout strategy: the GRU state is kept transposed in SBUF
(units on partitions, batch on the free dim) so all elementwise work
runs with 128 active partitions. The recurrent matmuls stream the
(bf16) recurrent weights through the PE array with the transposed
hidden state as the stationary operand; the [8, 3072] PSUM result is
escaped via ScalarE copies and transposed back to unit-major layout
with PE transposes.
"""

import numpy as np

B, T_FULL, U, E, V = 64, 512, 1024, 256, 32000
NCORES = 8
BL = B // NCORES          # 8 batch rows per core
TC = 32                   # scan steps per chunk
P = 128
UT = U // P               # 8 k-tiles over hidden units
GT = (3 * U) // P         # 24 tiles over gate units
NSL = 512                 # matmul free-dim slice

_BUILD_CACHE = {}


def _build(t_steps):
    import concourse.bass as bass
    import concourse.tile as tile
    import concourse.mybir as mybir
    from concourse import bacc
    from concourse.masks import make_identity

    nch = t_steps // TC
    F = (t_steps * BL) // P   # index columns

    nc = bacc.Bacc(None, target_bir_lowering=False, debug=False)
    dt = mybir.dt
    AF = mybir.ActivationFunctionType

    x_idx = nc.dram_tensor("idx", [P, F], dt.int32, kind="ExternalInput")
    h0 = nc.dram_tensor("h0", [P, UT * BL], dt.float32, kind="ExternalInput")
    emb = nc.dram_tensor("emb", [V, E], dt.bfloat16, kind="ExternalInput")
    wr = nc.dram_tensor("wr", [U, 3 * U], dt.bfloat16, kind="ExternalInput")
    wi = nc.dram_tensor("wi", [E, 3 * U], dt.bfloat16, kind="ExternalInput")
    bt = nc.dram_tensor("bt", [P, GT], dt.float32, kind="ExternalInput")
    y = nc.dram_tensor("y", [BL, t_steps, U], dt.float32, kind="ExternalOutput")

    with tile.TileContext(nc) as tc:
        with (
            tc.tile_pool(name="const", bufs=1) as const,
            tc.tile_pool(name="xt", bufs=2) as xt_pool,
            tc.tile_pool(name="win", bufs=2) as win_pool,
            tc.tile_pool(name="embp", bufs=2) as emb_pool,
            tc.tile_pool(name="sc8", bufs=2) as sc8,
            tc.tile_pool(name="scT", bufs=2) as scT,
            tc.tile_pool(name="flsh", bufs=3) as flsh,
            tc.tile_pool(name="pz", bufs=1, space="PSUM") as pz_pool,
            tc.tile_pool(name="ph", bufs=1, space="PSUM") as ph_pool,
            tc.tile_pool(name="pt", bufs=1, space="PSUM") as pt_pool,
            tc.tile_pool(name="pe", bufs=1, space="PSUM") as pe_pool,
        ):
            # ---- resident constants ----
            wr_sb = const.tile([P, UT, 3 * U], dt.bfloat16)
            nc.sync.dma_start(
                out=wr_sb[:], in_=wr[:].rearrange("(kt p) n -> p kt n", p=P)
            )
            wi_sb = const.tile([P, E // P, 3 * U], dt.bfloat16)
            nc.sync.dma_start(
                out=wi_sb[:], in_=wi[:].rearrange("(et p) n -> p et n", p=P)
            )
            bt_sb = const.tile([P, GT], dt.float32)
            nc.gpsimd.dma_start(out=bt_sb[:], in_=bt[:])
            idx_sb = const.tile([P, F], dt.int32)
            nc.gpsimd.dma_start(out=idx_sb[:], in_=x_idx[:])
            id_f32 = const.tile([P, P], dt.float32)
            make_identity(nc, id_f32[:])
            id_bf16 = const.tile([P, P], dt.bfloat16)
            nc.vector.tensor_copy(out=id_bf16[:], in_=id_f32[:])

            h_init = const.tile([P, UT, BL], dt.float32)
            nc.gpsimd.dma_start(
                out=h_init[:], in_=h0[:].rearrange("p (j b) -> p j b", b=BL)
            )

            RPC = TC * BL                 # gather rows per chunk (256)
            GPC = RPC // P                # gather tiles per chunk (2)

            prev_win = None
            hT_bf = None

            for c in range(nch):
                # ================= xproj phase for chunk c =================
                embT = emb_pool.tile([P, E // P, RPC], dt.bfloat16, tag="embT")
                for i in range(GPC):
                    g_tile = emb_pool.tile([P, E], dt.bfloat16, tag="gt")
                    col = c * GPC + i
                    nc.gpsimd.indirect_dma_start(
                        out=g_tile[:],
                        out_offset=None,
                        in_=emb[:],
                        in_offset=bass.IndirectOffsetOnAxis(
                            ap=idx_sb[:, col : col + 1], axis=0
                        ),
                    )
                    for et in range(E // P):
                        ps_e = pe_pool.tile([P, P], dt.bfloat16, tag="pe")
                        nc.tensor.transpose(
                            out=ps_e[:],
                            in_=g_tile[:, et * P : (et + 1) * P],
                            identity=id_bf16[:],
                        )
                        nc.vector.tensor_copy(
                            out=embT[:, et, i * P : (i + 1) * P], in_=ps_e[:]
                        )

                xT = xt_pool.tile([P, GT, TC, BL], dt.float32, tag="xT")
                for nt in range(GT):
                    ps_x = ph_pool.tile([P, RPC], dt.float32, tag="ph")
                    for et in range(E // P):
                        nc.tensor.matmul(
                            ps_x[:],
                            wi_sb[:, et, nt * P : (nt + 1) * P],
                            embT[:, et, :],
                            start=(et == 0),
                            stop=(et == E // P - 1),
                        )
                    nc.vector.tensor_scalar_add(
                        out=xT[:, nt],
                        in0=ps_x[:].rearrange("p (t b) -> p t b", b=BL),
                        scalar1=bt_sb[:, nt : nt + 1],
                    )

                # ================= scan steps for chunk c =================
                win = win_pool.tile([P, UT, BL, TC], dt.float32, tag="win")
                for s in range(TC):
                    t = c * TC + s
                    if t == 0:
                        h_prev = h_init[:]
                    elif s == 0:
                        h_prev = prev_win[:, :, :, TC - 1]
                    else:
                        h_prev = win[:, :, :, s - 1]
                    if hT_bf is None:
                        hT_bf = scT.tile([P, UT, BL], dt.bfloat16, tag="hbf")
                        nc.vector.tensor_copy(out=hT_bf[:], in_=h_prev)

                    # --- mm1: zr gates, r slices first ---
                    ps_zr = pz_pool.tile([BL, 2 * U], dt.float32, tag="pz")
                    for sl in (2, 3, 0, 1):
                        for kt in range(UT):
                            nc.tensor.matmul(
                                ps_zr[:, sl * NSL : (sl + 1) * NSL],
                                hT_bf[:, kt],
                                wr_sb[:, kt, sl * NSL : (sl + 1) * NSL],
                                start=(kt == 0),
                                stop=(kt == UT - 1),
                            )

                    ps_T = pt_pool.tile([P, 3 * UT * BL], dt.float32, tag="pt")

                    # --- r gate: escape, transpose, sigmoid ---
                    m_r = sc8.tile([BL, U], dt.float32, tag="mr")
                    nc.scalar.activation(
                        out=m_r[:], in_=ps_zr[:, U : 2 * U], func=AF.Copy
                    )
                    for j in range(UT):
                        nc.tensor.transpose(
                            out=ps_T[:, j * BL : (j + 1) * BL],
                            in_=m_r[:, j * P : (j + 1) * P],
                            identity=id_f32[:BL, :BL],
                        )
                    tmp_r = scT.tile([P, UT, BL], dt.float32, tag="tr")
                    nc.vector.tensor_add(
                        out=tmp_r[:],
                        in0=ps_T[:, 0 : UT * BL].rearrange("p (j b) -> p j b", b=BL),
                        in1=xT[:, UT : 2 * UT, s],
                    )
                    rT = scT.tile([P, UT, BL], dt.float32, tag="rT")
                    nc.scalar.activation(out=rT[:], in_=tmp_r[:], func=AF.Sigmoid)
                    rhT_bf = scT.tile([P, UT, BL], dt.bfloat16, tag="rhbf")
                    nc.vector.tensor_mul(out=rhT_bf[:], in0=rT[:], in1=h_prev)

                    # --- mm2: candidate ---
                    ps_h = ph_pool.tile([BL, U], dt.float32, tag="ph")
                    for sl in range(2):
                        for kt in range(UT):
                            nc.tensor.matmul(
                                ps_h[:, sl * NSL : (sl + 1) * NSL],
                                rhT_bf[:, kt],
                                wr_sb[:, kt, 2 * U + sl * NSL : 2 * U + (sl + 1) * NSL],
                                start=(kt == 0),
                                stop=(kt == UT - 1),
                            )

                    # --- z gate (overlaps mm2) ---
                    m_z = sc8.tile([BL, U], dt.float32, tag="mz")
                    nc.scalar.activation(
                        out=m_z[:], in_=ps_zr[:, 0:U], func=AF.Copy
                    )
                    for j in range(UT):
                        nc.tensor.transpose(
                            out=ps_T[:, UT * BL + j * BL : UT * BL + (j + 1) * BL],
                            in_=m_z[:, j * P : (j + 1) * P],
                            identity=id_f32[:BL, :BL],
                        )
                    tmp_z = scT.tile([P, UT, BL], dt.float32, tag="tz")
                    nc.vector.tensor_add(
                        out=tmp_z[:],
                        in0=ps_T[:, UT * BL : 2 * UT * BL].rearrange(
                            "p (j b) -> p j b", b=BL
                        ),
                        in1=xT[:, 0:UT, s],
                    )
                    zT = scT.tile([P, UT, BL], dt.float32, tag="zT")
                    nc.scalar.activation(out=zT[:], in_=tmp_z[:], func=AF.Sigmoid)

                    # --- candidate: escape, transpose, tanh ---
                    m_h = sc8.tile([BL, U], dt.float32, tag="mh")
                    nc.scalar.activation(out=m_h[:], in_=ps_h[:], func=AF.Copy)
                    for j in range(UT):
                        nc.tensor.transpose(
                            out=ps_T[:, 2 * UT * BL + j * BL : 2 * UT * BL + (j + 1) * BL],
                            in_=m_h[:, j * P : (j + 1) * P],
                            identity=id_f32[:BL, :BL],
                        )
                    tmp_h = scT.tile([P, UT, BL], dt.float32, tag="th")
                    nc.vector.tensor_add(
                        out=tmp_h[:],
                        in0=ps_T[:, 2 * UT * BL : 3 * UT * BL].rearrange(
                            "p (j b) -> p j b", b=BL
                        ),
                        in1=xT[:, 2 * UT : 3 * UT, s],
                    )
                    hhT = scT.tile([P, UT, BL], dt.float32, tag="hhT")
                    nc.scalar.activation(out=hhT[:], in_=tmp_h[:], func=AF.Tanh)

                    # --- h update: h' = hh + z * (h - hh) ---
                    d = scT.tile([P, UT, BL], dt.float32, tag="d")
                    nc.vector.tensor_sub(out=d[:], in0=h_prev, in1=hhT[:])
                    zd = scT.tile([P, UT, BL], dt.float32, tag="zd")
                    nc.vector.tensor_mul(out=zd[:], in0=zT[:], in1=d[:])
                    nc.vector.tensor_add(out=win[:, :, :, s], in0=hhT[:], in1=zd[:])
                    hT_bf = scT.tile([P, UT, BL], dt.bfloat16, tag="hbf")
                    nc.vector.tensor_copy(out=hT_bf[:], in_=win[:, :, :, s])

                # ================= flush window c to HBM =================
                t0 = c * TC
                for j in range(UT):
                    for b0 in range(0, BL, 4):
                        ps_o = pe_pool.tile([P, P], dt.float32, tag="pe")
                        nc.tensor.transpose(
                            out=ps_o[:],
                            in_=win[:, j, b0 : b0 + 4, :].rearrange(
                                "p b t -> p (b t)"
                            ),
                            identity=id_f32[:],
                        )
                        stage = flsh.tile([P, P], dt.float32, tag="stage")
                        nc.vector.tensor_copy(out=stage[:], in_=ps_o[:])
                        for g in range(4):
                            nc.sync.dma_start(
                                out=y[b0 + g, t0 : t0 + TC, j * P : (j + 1) * P],
                                in_=stage[g * TC : (g + 1) * TC, :],
                            )
                prev_win = win

    nc.finalize()
    return nc


def _host_pack(x, hidden, emb_table, kern, rec_kernel, bias):
    import ml_dtypes

    t_steps = x.shape[1]
    F = (t_steps * BL) // P
    emb_bf = np.asarray(emb_table, np.float32).astype(ml_dtypes.bfloat16)
    wr_bf = np.asarray(rec_kernel, np.float32).astype(ml_dtypes.bfloat16)
    wi_bf = np.asarray(kern, np.float32).astype(ml_dtypes.bfloat16)
    bt = np.asarray(bias, np.float32).reshape(GT, P).T.copy()  # [128, 24]

    in_maps = []
    for c in range(NCORES):
        xs = np.asarray(x[c * BL : (c + 1) * BL], np.int32)        # [8, T]
        hs = np.asarray(hidden[c * BL : (c + 1) * BL], np.float32)  # [8, U]
        rows = xs.T.reshape(-1)                                    # r = t*BL + b
        idx = rows.reshape(F, P).T.copy()                          # [128, F]
        h0 = hs.reshape(BL, UT, P).transpose(2, 1, 0).reshape(P, UT * BL)
        in_maps.append(
            {
                "idx": idx,
                "h0": np.ascontiguousarray(h0),
                "emb": emb_bf,
                "wr": wr_bf,
                "wi": wi_bf,
                "bt": np.ascontiguousarray(bt),
            }
        )
    return in_maps


def kernel(x, hidden, emb_table, kernel, recurrent_kernel, bias):
    from concourse.bass_utils import run_bass_kernel_spmd

    x = np.asarray(x)
    t_steps = x.shape[1]
    if t_steps not in _BUILD_CACHE:
        _BUILD_CACHE[t_steps] = _build(t_steps)
    nc = _BUILD_CACHE[t_steps]

    in_maps = _host_pack(x, hidden, emb_table, kernel, recurrent_kernel, bias)
    res = run_bass_kernel_spmd(nc, in_maps, core_ids=list(range(NCORES)))
    out = np.concatenate([r["y"] for r in res.results], axis=0)
    state = out[:, -1, :].copy()
    return out, state
